# revision 1
# baseline (speedup 1.0000x reference)
"""Chamfer loss Trainium2 kernel.

Problem: pred/target [8, 4096, 3] fp32. loss = (mean_n min_m d + mean_m min_n d)/2,
d = relu(|p|^2 + |t|^2 - 2 p.t).

Sharding: one batch per NeuronCore (8 cores).

Per-core algorithm:
  Let md[n,mi] = p.t - p2/2 - t2/2  (= -d/2). Then
    dist1[n] = relu(-2 * max_mi md[n,:])   and symmetrically for dist2.
  md is computed EXACTLY (to fp32) on the TensorEngine as a single K=33 bf16
  matmul per [128,512] tile: coordinates are split into 3 bf16 components
  (h+m+l captures the full fp32 mantissa); all 9 split-pair products are
  exact in bf16*bf16->fp32 PSUM accumulation. The -p2/2 / -t2/2 terms ride
  along as extra K rows against constant-one rows.

  Row k = 9a + 3d + b (a = pred split class, b = target split class,
  d = coordinate):  A[k] = split_a(p)[d],  B[k] = split_b(t)[d];
  rows 27-29: A = -p2/2 splits, B = ones; rows 30-32: A = ones, B = -t2/2
  splits.  dir1 tile = A_blk.T @ B_chunk (rows=pred, free=target);
  dir2 tile = B_blk.T @ A_chunk.  A and B are duplicated at partition 64 so
  consecutive matmuls of an n-tile PAIR hit different PE row groups, letting
  the PE's reorder window overlap each LDWEIGHTS with the previous matmul.

  Row-max over the 4096-wide free dim: VectorE tensor_tensor_scan with
  op0=op1=max consumes a PSUM half and an ACT-copied SBUF half at
  2 elements/cycle; the last scan element is the row max (chained across
  halves via the scalar initial).

Inputs are pre-transposed on the HOST (predT/targT [3,4096] and the natural
[128,96] view) so every DMA load is contiguous.
"""

import numpy as np
from contextlib import ExitStack

N = 4096  # points per cloud
B = 8     # batches == cores
NT = N // 128  # 32 n-tiles
HALF = 2048    # m-range covered per (pa, pb) psum pair
QUART = 1024   # psum tile free size

_CACHE = {}


def _emit(tc, nc, mybir, predT, targT, predQ, targQ, out_dram,
          reps=None, variant=None):
    f32 = mybir.dt.float32
    bf16 = mybir.dt.bfloat16
    Alu = mybir.AluOpType
    Act = mybir.ActivationFunctionType
    Axis = mybir.AxisListType

    from concourse.bass import _add_dep_helper

    with ExitStack() as ctx:
        const = ctx.enter_context(tc.tile_pool(name="const", bufs=1))
        psum = ctx.enter_context(tc.tile_pool(name="psum", bufs=1, space="PSUM"))
        sbcopy = ctx.enter_context(tc.tile_pool(name="sbcopy", bufs=3))
        scratch = ctx.enter_context(tc.tile_pool(name="scratch", bufs=4))

        def body():
            # ---------------- prep: load + split ----------------
            # x6 rows: 0-2 pred xyz, 3-5 target xyz (natural point order)
            x6 = const.tile([6, N], f32)
            nc.sync.dma_start(x6[0:3, :], predT)
            nc.sync.dma_start(x6[3:6, :], targT)

            # contiguous loads for p2/t2 ([128, 96] = [128, 32 points, 3])
            xp96 = const.tile([128, 96], f32)
            xt96 = const.tile([128, 96], f32)
            nc.sync.dma_start(xp96[:], predQ)
            nc.sync.dma_start(xt96[:], targQ)

            # 3-way bf16 split of coordinates: x = h + m + l (exact to fp32).
            # Classes live in free-dim slices of ONE tile so one DMA can
            # gather them: HML[:, c*N:(c+1)*N] = class c; rows 0-2 pred,
            # 3-5 target.
            HML = const.tile([6, 3 * N], bf16)
            r1 = const.tile([6, N], f32)
            r2 = const.tile([6, N], f32)
            nc.vector.tensor_copy(HML[:, 0:N], x6[:])
            nc.vector.tensor_sub(r1[:], x6[:], HML[:, 0:N])
            nc.vector.tensor_copy(HML[:, N : 2 * N], r1[:])
            nc.vector.tensor_sub(r2[:], r1[:], HML[:, N : 2 * N])
            hml_done = nc.vector.tensor_copy(HML[:, 2 * N : 3 * N], r2[:])

            # -p2/2, -t2/2 in [128, 64] layout (cols 0:32 pred, 32:64
            # target), then 3-way split.  Point (32p + q) lives at [p, q].
            sqp = const.tile([128, 96], f32)
            sqt = const.tile([128, 96], f32)
            nc.vector.tensor_mul(sqp[:], xp96[:], xp96[:])
            nc.vector.tensor_mul(sqt[:], xt96[:], xt96[:])
            pt2 = const.tile([128, 64], f32)
            nc.vector.tensor_reduce(
                pt2[:, 0:32], sqp[:].rearrange("p (q d) -> p q d", d=3),
                axis=Axis.X, op=Alu.add,
            )
            nc.vector.tensor_reduce(
                pt2[:, 32:64], sqt[:].rearrange("p (q d) -> p q d", d=3),
                axis=Axis.X, op=Alu.add,
            )
            pt2n = const.tile([128, 64], f32)
            nc.vector.tensor_scalar_mul(pt2n[:], pt2[:], -0.5)
            # Q[:, 64c:64c+64] = split class c of (-p2/2 | -t2/2); cols 0:32
            # within a class = pred, 32:64 = target.
            Q = const.tile([128, 192], bf16)
            q_r1 = const.tile([128, 64], f32)
            q_r2 = const.tile([128, 64], f32)
            nc.vector.tensor_copy(Q[:, 0:64], pt2n[:])
            nc.vector.tensor_sub(q_r1[:], pt2n[:], Q[:, 0:64])
            nc.vector.tensor_copy(Q[:, 64:128], q_r1[:])
            nc.vector.tensor_sub(q_r2[:], q_r1[:], Q[:, 64:128])
            q_done = nc.vector.tensor_copy(Q[:, 128:192], q_r2[:])

            # ------------- assemble A / B [33, 4096] bf16 (+ dup @64) ------
            A = const.tile([97, N], bf16)
            Bm = const.tile([97, N], bf16)
            assembly = []
            for a in range(3):
                a_src = (
                    HML[0:3, a * N : (a + 1) * N]
                    .unsqueeze(1)
                    .broadcast_to((3, 3, N))
                )  # dims (d, b broadcast, j)
                b_src = HML[3:6, :].rearrange("p (b j) -> p b j", b=3)
                da = nc.sync.dma_start(A[9 * a : 9 * a + 9, :], a_src)
                db = nc.sync.dma_start(Bm[9 * a : 9 * a + 9, :], b_src)
                for dd in (da, db):
                    _add_dep_helper(
                        dd.ins, hml_done.ins, sync=True, reason="hml ready"
                    )
                assembly += [da, db]
            for j in range(3):
                da = nc.sync.dma_start(
                    A[27 + j : 28 + j, :], Q[:, 64 * j : 64 * j + 32]
                )
                db = nc.sync.dma_start(
                    Bm[30 + j : 31 + j, :], Q[:, 64 * j + 32 : 64 * j + 64]
                )
                for dd in (da, db):
                    _add_dep_helper(
                        dd.ins, q_done.ins, sync=True, reason="q ready"
                    )
                assembly += [da, db]
            ones3 = const.tile([3, N], bf16)
            ones_set = nc.vector.memset(ones3[:], 1.0)
            da = nc.sync.dma_start(A[30:33, :], ones3[:])
            db = nc.sync.dma_start(Bm[27:30, :], ones3[:])
            for dd in (da, db):
                _add_dep_helper(dd.ins, ones_set.ins, sync=True, reason="ones")
            assembly += [da, db]
            # duplicates at partition 64 for row-group interleaving
            dupA = nc.sync.dma_start(A[64:97, :], A[0:33, :])
            dupB = nc.sync.dma_start(Bm[64:97, :], Bm[0:33, :])
            for dd in assembly[:]:
                _add_dep_helper(dupA.ins, dd.ins, sync=True, reason="dupA")
                _add_dep_helper(dupB.ins, dd.ins, sync=True, reason="dupB")
            dups = [dupA, dupB]

            if variant == "prep":
                nc.sync.dma_start(out_dram[:], pt2n[:, 0:2])
                return

            # ---------------- main loop ----------------
            # cols 0:32 dir1, 32:64 dir2
            partials = const.tile([128, 2 * NT], f32)

            def scan_site(pa, sb, dump, init):
                if variant == "mmonly":
                    pass
                elif variant == "noscan":
                    nc.vector.memset(dump[:, QUART - 1 : QUART], 0.0)
                else:
                    nc.vector.tensor_tensor_scan(
                        out=dump[:], data0=pa[:], data1=sb[:], initial=init,
                        op0=Alu.max, op1=Alu.max,
                    )

            first_mm = True
            for dr in range(2):
                lhs_mat, rhs_mat = (A, Bm) if dr == 0 else (Bm, A)
                for ip in range(NT // 2):
                    i0, i1 = 2 * ip, 2 * ip + 1
                    # row-group offset per pair member: i0 -> rows 0:33,
                    # i1 -> rows 64:97
                    lhs0 = lhs_mat[0:33, i0 * 128 : (i0 + 1) * 128]
                    lhs1 = lhs_mat[64:97, i1 * 128 : (i1 + 1) * 128]
                    rhs0 = rhs_mat[0:33, :]
                    rhs1 = rhs_mat[64:97, :]
                    prev = [None, None]
                    for half in range(2):
                        base = half * HALF
                        pa0 = psum.tile([128, QUART], f32, tag="pa0")
                        pb0 = psum.tile([128, QUART], f32, tag="pb0")
                        pa1 = psum.tile([128, QUART], f32, tag="pa1")
                        pb1 = psum.tile([128, QUART], f32, tag="pb1")
                        # interleave row groups so LDWEIGHTS overlaps MMs;
                        # pb first so ACT copies can start early
                        for c, lo in ((0, 1024), (1, 1536)):
                            mm = nc.tensor.matmul(
                                pb0[:, c * 512 : c * 512 + 512],
                                lhs0, rhs0[:, base + lo : base + lo + 512],
                            )
                            if first_mm:
                                for dd in assembly:
                                    _add_dep_helper(
                                        mm.ins, dd.ins, sync=True,
                                        reason="mats ready",
                                    )
                            mm1 = nc.tensor.matmul(
                                pb1[:, c * 512 : c * 512 + 512],
                                lhs1, rhs1[:, base + lo : base + lo + 512],
                            )
                            if first_mm:
                                for dd in dups:
                                    _add_dep_helper(
                                        mm1.ins, dd.ins, sync=True,
                                        reason="dups ready",
                                    )
                                first_mm = False
                        for c, lo in ((0, 0), (1, 512)):
                            nc.tensor.matmul(
                                pa0[:, c * 512 : c * 512 + 512],
                                lhs0, rhs0[:, base + lo : base + lo + 512],
                            )
                            nc.tensor.matmul(
                                pa1[:, c * 512 : c * 512 + 512],
                                lhs1, rhs1[:, base + lo : base + lo + 512],
                            )
                        sb0 = sbcopy.tile([128, QUART], f32, tag="sb0")
                        sb1 = sbcopy.tile([128, QUART], f32, tag="sb1")
                        if variant != "mmonly":
                            nc.scalar.copy(sb0[:], pb0[:])
                            nc.scalar.copy(sb1[:], pb1[:])
                        d0 = scratch.tile([128, QUART], f32, tag="d0")
                        d1 = scratch.tile([128, QUART], f32, tag="d1")
                        init0 = -1e30 if half == 0 else prev[0][:, QUART - 1 :]
                        init1 = -1e30 if half == 0 else prev[1][:, QUART - 1 :]
                        scan_site(pa0, sb0, d0, init0)
                        scan_site(pa1, sb1, d1, init1)
                        prev = [d0, d1]
                    # last scan element = row max over all 4096 m
                    for sl, i in ((0, i0), (1, i1)):
                        col = dr * NT + i
                        if variant in ("mmonly", "noscan"):
                            nc.vector.memset(partials[:, col : col + 1], 0.0)
                        else:
                            nc.scalar.copy(
                                partials[:, col : col + 1],
                                prev[sl][:, QUART - 1 : QUART],
                            )

            # ---------------- finals ----------------
            # dist = relu(-2 * maxm); sum the 32 n-tile columns per direction
            relu = const.tile([128, 2 * NT], f32)
            nc.scalar.activation(relu[:], partials[:], Act.Relu, scale=-2.0)
            sums = const.tile([128, 2], f32)
            nc.vector.tensor_reduce(
                sums[:, 0:1], relu[:, 0:NT], axis=Axis.X, op=Alu.add
            )
            nc.vector.tensor_reduce(
                sums[:, 1:2], relu[:, NT : 2 * NT], axis=Axis.X, op=Alu.add
            )
            nc.sync.dma_start(out_dram[:], sums[:])

        if reps is None or reps <= 1:
            body()
        else:
            with tc.For_i(0, reps, 1):
                body()


def build_bass(reps=None, variant=None):
    import concourse.tile as tile
    from concourse import bacc, mybir

    f32 = mybir.dt.float32
    nc = bacc.Bacc("TRN2", target_bir_lowering=False, debug=False, num_devices=B)
    predT = nc.dram_tensor("predT", [3, N], f32, kind="ExternalInput").ap()
    targT = nc.dram_tensor("targT", [3, N], f32, kind="ExternalInput").ap()
    predQ = nc.dram_tensor("predQ", [128, 96], f32, kind="ExternalInput").ap()
    targQ = nc.dram_tensor("targQ", [128, 96], f32, kind="ExternalInput").ap()
    out = nc.dram_tensor("out", [128, 2], f32, kind="ExternalOutput").ap()
    with tile.TileContext(nc) as tc:
        _emit(tc, nc, mybir, predT, targT, predQ, targQ, out,
              reps=reps, variant=variant)
    nc.compile()
    return nc


def _get_nc():
    if "nc" not in _CACHE:
        _CACHE["nc"] = build_bass()
    return _CACHE["nc"]


def make_in_maps(pred, target):
    maps = []
    for b in range(B):
        p = np.ascontiguousarray(pred[b], dtype=np.float32)
        t = np.ascontiguousarray(target[b], dtype=np.float32)
        maps.append(
            {
                "predT": np.ascontiguousarray(p.T),
                "targT": np.ascontiguousarray(t.T),
                "predQ": p.reshape(128, 96),
                "targQ": t.reshape(128, 96),
            }
        )
    return maps


def kernel(pred: np.ndarray, target: np.ndarray) -> np.ndarray:
    from concourse.bass_utils import run_bass_kernel_spmd

    nc = _get_nc()
    res = run_bass_kernel_spmd(nc, make_in_maps(pred, target),
                               core_ids=list(range(B)))
    s1 = 0.0
    s2 = 0.0
    for b in range(B):
        o = res.results[b]["out"].astype(np.float64)
        s1 += o[:, 0].sum()
        s2 += o[:, 1].sum()
    loss = (s1 / (B * N) + s2 / (B * N)) / 2.0
    return np.float32(loss)

